# revision 11
# baseline (speedup 1.0000x reference)
"""Multi-head self-attention (B=4, S=2048, D=1024, H=16) on 8 trn2 NeuronCores.

Sharding: data-parallel over (batch, seq-half) -> 8 shards of 1024 query rows.
K/V halves are exchanged between the two cores of a batch via pairwise
AllGather collectives (each core projects K/V only for its own 1024 rows).

Per-core device kernel (Tile / fp32r matmuls, bf16 for AV + output proj):
  xT = query[b].T (host-transposed, rolled so this core's q rows are cols 0:1023)
  kT[g]  = (Wk x_own)^T : [128 dout, 1024 kr] own half, AllGather -> 2048 kr
  qT[g]  = (Wq x)^T   : [128 dout, 1024 qr]
  v      = x_own^T Wv^T : [kr, dout] + ones column per head (bf16), AllGather
  scores^T = kT^T qT  : [kr, qr] per (head, kr-tile)
  exp: ACT spline for most kr-tiles; Schraudolph bf16 bit-trick on DVE for
       CFG["dve_t"] of 16 tiles (offloads the ACT bottleneck)
  attn_outT/denom     = v_aug^T exp : [65, qr] PSUM (M=65: 64 hd + denom row)
  normalize: 3-op Newton reciprocal on DVE (int-seed + 1 NR), negated-ones
       outer-product broadcast on PE, DVE multiply
  out = attn_outT^T WoT -> [1024 qr, 1024 dout] fp32
"""
import sys

sys.path.insert(0, "/opt/trn_rl_repo")

import numpy as np

B, S, D, H = 4, 2048, 1024, 16
HD = D // H          # 64
P = 128
Q = S // 2           # q rows per core
NG = H // 2          # 8 head-pair groups
DT = D // P          # 8 din tiles
KT = S // P          # 16 kr tiles
SCALE = 1.0 / np.sqrt(np.float32(HD))  # 0.125

# Schraudolph exp -> bf16 bits: round(x*SCALE*128/ln2 + (127*128 - 7))
A_SCH = float(0.125 * 128.0 / np.log(2.0))
B_SCH = float(127.0 * 128.0 - 7.0)
# Newton reciprocal seed: bits = NOT(x_bits) + RC_C (== RC_C - 1 - x_bits)
RC_C = 0xFEF327EB  # (C1+1) for seed of -1/x

_CACHE = {}

# tunables
CFG = {
    "kt_bufs": 3, "qt_bufs": 2, "v_bufs": KT + 1, "exp_bufs": 4,
    "sc_bufs": 2, "av_bufs": 2, "pr_bufs": 2, "rc_bufs": 2,
    "v_inter": True,
    "kv_exchange": True,
    "dve_t": 4,           # how many of 16 kr-tiles use the DVE bit-trick exp
    "nr_recip": True,
    "act_evict": ("v", "out"),  # which psum evictions run on ACT
}
import os as _os, json as _json
if _os.environ.get("KCFG"):
    CFG.update(_json.loads(_os.environ["KCFG"]))


def _build_bass(repeat=1):
    import concourse.bass as bass
    import concourse.tile as tile
    from concourse import mybir
    from concourse.vector_clock import ScopedClock

    f32 = mybir.dt.float32
    f32r = mybir.dt.float32r
    bf16 = mybir.dt.bfloat16
    i32 = mybir.dt.int32
    i16 = mybir.dt.int16

    # This walrus build only accepts ONE sync-wait per CTRL instruction; the
    # stock Tile exit drain packs all outstanding sem waits onto a single
    # Drain. Spread them across sync-engine nops instead.
    def _drain_and_barrier(self, tick_clock, wait_clock):
        nc = self.nc
        drain_inst = nc.sync.drain()
        wait_clock.add_sem_waits(
            drain_inst.ins, ScopedClock({None: tick_clock.global_clock})
        )
        si = drain_inst.ins.sync_info
        waits = list(si.on_wait) if si is not None else []
        if len(waits) > 1:
            drain_inst.ins.sync_info = mybir.SyncInfo(
                on_wait=waits[:1], on_update=list(si.on_update)
            )
            for i in range(1, len(waits)):
                nop = nc.sync.nop(nofuse=True, hint="drain_wait_split")
                nop.ins.sync_info = mybir.SyncInfo(on_wait=[waits[i]], on_update=[])
        nc.all_engine_barrier()
        popped = nc._tile_sem_poison_stack.pop()
        assert popped is self._sem_poison
        nc.clear_and_free_semaphores(list(self.sems.allocated().values()))
        nc.all_engine_barrier()

    tile.TileContext._drain_and_barrier = _drain_and_barrier

    # Same walrus limitation, general case: any instruction may carry at most
    # one sync wait. Split extras onto same-engine nops placed just before.
    if not getattr(tile.TileContext, "_wait_split_patched", False):
        _orig_lower = tile.TileContext._lower_ordered_insts

        def _lower_with_wait_split(self, ordered):
            counter = 0
            for bb_name in list(ordered.keys()):
                new_insts = []
                for inst in ordered[bb_name]:
                    si = inst.sync_info
                    if si is not None and len(si.on_wait or []) > 1:
                        waits = list(si.on_wait)
                        for w in waits[:-1]:
                            counter += 1
                            nop = mybir.InstNoOp(
                                name=f"I-waitsplit-{bb_name}-{counter}", ins=[], outs=[]
                            )
                            nop.engine = inst.engine
                            nop.sync_info = mybir.SyncInfo(on_wait=[w], on_update=[])
                            new_insts.append(nop)
                        inst.sync_info = mybir.SyncInfo(
                            on_wait=[waits[-1]], on_update=list(si.on_update)
                        )
                    new_insts.append(inst)
                ordered[bb_name] = new_insts
            return _orig_lower(self, ordered)

        tile.TileContext._lower_ordered_insts = _lower_with_wait_split
        tile.TileContext._wait_split_patched = True

    Exp = mybir.ActivationFunctionType.Exp

    nc = bass.Bass()
    xT_d = nc.dram_tensor("xT", [D, S], f32r, kind="ExternalInput")
    wqT_d = nc.dram_tensor("wqT", [D, D], f32r, kind="ExternalInput")
    wkT_d = nc.dram_tensor("wkT", [D, D], f32r, kind="ExternalInput")
    wvT_d = nc.dram_tensor("wvT", [D, D], f32r, kind="ExternalInput")
    woT_d = nc.dram_tensor("woT", [D, D], bf16, kind="ExternalInput")
    out_d = nc.dram_tensor("out", [Q, D], f32, kind="ExternalOutput")

    from contextlib import ExitStack

    with tile.TileContext(nc) as tc:
        with ExitStack() as ctx:
            pool = lambda name, bufs, **kw: ctx.enter_context(
                tc.tile_pool(name=name, bufs=bufs, **kw)
            )
            xr_p = pool("xr", DT)            # xT fp32r resident
            wqk_p = pool("wqk", 10)          # per-group W tiles
            wv_p = pool("wv", 8)
            wo_p = pool("wo", 9)
            kt_p = pool("kt", CFG["kt_bufs"])
            kto_p = pool("kto", 1)           # own-half kT staging
            qt_p = pool("qt", CFG["qt_bufs"])
            v_p = pool("vv", CFG["v_bufs"])
            vo_p = pool("vo", 8)             # own-half v staging
            exp_p = pool("expp", CFG["exp_bufs"])
            attn_p = pool("attn", DT)
            odd_p = pool("odd", 2)
            rc_p = pool("rc", CFG["rc_bufs"])
            bcb_p = pool("bcb", 2)
            out_p = pool("outp", 2)
            ones_p = pool("ones", 1)
            dram_p = pool("dramb", 24, space="DRAM") if CFG["kv_exchange"] else None
            ps_sc = pool("ps_sc", CFG["sc_bufs"], space="PSUM")
            ps_av = pool("ps_av", CFG["av_bufs"], space="PSUM")
            ps_pr = pool("ps_pr", CFG["pr_bufs"], space="PSUM")
            # negated-ones row for the reciprocal-broadcast outer product
            # (NR recip produces -1/d; -1 * -1/d = +1/d)
            ones_f = ones_p.tile([P, HD], f32, tag="ones_f")
            nc.vector.memset(ones_f[:], -1.0 if CFG["nr_recip"] else 1.0)
            ones_r = ones_p.tile([P, HD], f32r, tag="ones_r")
            nc.vector.tensor_copy(ones_r[:], ones_f[:])
            seed_c = None  # reciprocal seed constants passed as immediates

            for _rep in range(repeat):
                _kernel_body(
                    nc, tc, mybir, f32, f32r, bf16, i32, i16, Exp,
                    xT_d, wqT_d, wkT_d, wvT_d, woT_d, out_d,
                    xr_p, wqk_p, wv_p, wo_p, kt_p, kto_p, qt_p, v_p, vo_p,
                    exp_p, attn_p, odd_p, rc_p, bcb_p, out_p, dram_p,
                    ones_r, seed_c, ps_sc, ps_av, ps_pr,
                )
    return nc


def _kernel_body(
    nc, tc, mybir, f32, f32r, bf16, i32, i16, Exp,
    xT_d, wqT_d, wkT_d, wvT_d, woT_d, out_d,
    xr_p, wqk_p, wv_p, wo_p, kt_p, kto_p, qt_p, v_p, vo_p,
    exp_p, attn_p, odd_p, rc_p, bcb_p, out_p, dram_p,
    ones_r, seed_c, ps_sc, ps_av, ps_pr,
):
    from collections import deque

    Alu = mybir.AluOpType
    exchange = CFG["kv_exchange"]
    dve_tset = set()
    if CFG["dve_t"] > 0:
        step = KT // CFG["dve_t"]
        dve_tset = {(i * step + step // 2) % KT for i in range(CFG["dve_t"])}

    def _evict(kind, dst, src):
        if kind in CFG["act_evict"]:
            nc.scalar.copy(dst, src)
        else:
            nc.vector.tensor_copy(dst, src)

    # load xT (pre-rounded fp32r bits) straight into resident tiles.
    # Own-half (cols 0:1024) first so own-half K/V projection starts early.
    xr = [xr_p.tile([P, S], f32r, tag="xr", name=f"xr{dt}") for dt in range(DT)]
    for ch in range(4):
        for dt in range(DT):
            nc.sync.dma_start(
                xr[dt][:, ch * 512:(ch + 1) * 512],
                xT_d[dt * P:(dt + 1) * P, ch * 512:(ch + 1) * 512],
            )

    state = {"v_sb": [None] * KT, "kt": {}, "qt": {}}

    def _proj_steps(g):
        """Emit-steps producing qt[g], kt[g] (+exchange) and v superblock."""
        steps = []
        wq_r, wk_r = [], []

        def load_w():
            for dt in range(DT):
                for w_d, lst, tg in ((wqT_d, wq_r, "wq"), (wkT_d, wk_r, "wk")):
                    wr = wqk_p.tile([P, P], f32r, tag=tg)
                    nc.sync.dma_start(
                        wr[:], w_d[dt * P:(dt + 1) * P, g * P:(g + 1) * P]
                    )
                    lst.append(wr)
        steps.append(load_w)

        qt = qt_p.tile([P, Q], f32r, tag="qt", name=f"qt{g}")
        state["qt"][g] = qt

        def qt_step(n):
            def _s():
                ps = ps_pr.tile([P, 512], f32, tag="prj")
                for dt in range(DT):
                    nc.tensor.matmul(
                        ps[:], wq_r[dt][:], xr[dt][:, n * 512:(n + 1) * 512],
                        start=(dt == 0), stop=(dt == DT - 1),
                    )
                _evict("qt", qt[:, n * 512:(n + 1) * 512], ps[:])
            return _s
        steps += [qt_step(0), qt_step(1)]

        kt = kt_p.tile([P, S], f32r, tag="kt", name=f"kt{g}")
        state["kt"][g] = kt
        if exchange:
            kto = kto_p.tile([P, Q], f32r, tag="kto", name=f"kto{g}")

            def kt_step(n):
                def _s():
                    ps = ps_pr.tile([P, 512], f32, tag="prj")
                    for dt in range(DT):
                        nc.tensor.matmul(
                            ps[:], wk_r[dt][:], xr[dt][:, n * 512:(n + 1) * 512],
                            start=(dt == 0), stop=(dt == DT - 1),
                        )
                    _evict("kt", kto[:, n * 512:(n + 1) * 512], ps[:])
                return _s

            def kt_ag():
                kin = dram_p.tile([P, Q], f32r, tag="kin", name=f"kin{g}")
                kout = dram_p.tile([2 * P, Q], f32r, tag="kout", name=f"kout{g}")
                nc.gpsimd.dma_start(kin[:], kto[:])
                nc.gpsimd.collective_compute(
                    "AllGather",
                    Alu.bypass,
                    replica_groups=[[0, 1], [2, 3], [4, 5], [6, 7]],
                    ins=[kin.opt()],
                    outs=[kout.opt()],
                )
                nc.sync.dma_start(kt[:, 0:Q], kout[0:P, :])
                nc.sync.dma_start(kt[:, Q:S], kout[P:2 * P, :])
            steps += [kt_step(0), kt_step(1), kt_ag]
        else:
            def kt_step(n):
                def _s():
                    ps = ps_pr.tile([P, 512], f32, tag="prj")
                    for dt in range(DT):
                        nc.tensor.matmul(
                            ps[:], wk_r[dt][:], xr[dt][:, n * 512:(n + 1) * 512],
                            start=(dt == 0), stop=(dt == DT - 1),
                        )
                    _evict("kt", kt[:, n * 512:(n + 1) * 512], ps[:])
                return _s
            steps += [kt_step(n) for n in range(4)]

        # V projection for superblock of 8 heads (every 4 groups)
        if g % 4 == 0:
            sbi = g // 4
            wv_r = []

            def load_wv():
                for dt in range(DT):
                    wr = wv_p.tile([P, 512], f32r, tag="wv")
                    nc.sync.dma_start(
                        wr[:],
                        wvT_d[dt * P:(dt + 1) * P, sbi * 512:(sbi + 1) * 512],
                    )
                    wv_r.append(wr)
            steps.append(load_wv)

            def vproj_step(t, dst_pool, dst_list, idx):
                def _s():
                    ps = ps_pr.tile([P, 512], f32, tag="prj", name=f"vps{g}_{t}")
                    for dt in range(DT):
                        nc.tensor.matmul(
                            ps[:], xr[dt][:, t * P:(t + 1) * P], wv_r[dt][:],
                            start=(dt == 0), stop=(dt == DT - 1),
                        )
                    vt = dst_pool.tile(
                        [P, 8 * (HD + 1)], bf16, tag="v", name=f"v{g}_{t}"
                    )
                    dst = vt[:].rearrange("p (h c) -> p h c", c=HD + 1)
                    src = ps[:].rearrange("p (h c) -> p h c", c=HD)
                    _evict("v", dst[:, :, 0:HD], src[:])
                    nc.vector.memset(dst[:, :, HD:HD + 1], 1.0)
                    dst_list[idx] = vt
                return _s

            if exchange:
                vown = [None] * (KT // 2)
                for tl in range(KT // 2):
                    steps.append(vproj_step(tl, vo_p, vown, tl))

                def v_ag():
                    vin = dram_p.tile(
                        [Q, 8 * (HD + 1)], bf16, tag="vin", name=f"vin{sbi}"
                    )
                    vout = dram_p.tile(
                        [S, 8 * (HD + 1)], bf16, tag="vout", name=f"vout{sbi}"
                    )
                    for tl in range(KT // 2):
                        nc.gpsimd.dma_start(
                            vin[tl * P:(tl + 1) * P, :], vown[tl][:]
                        )
                    nc.gpsimd.collective_compute(
                        "AllGather",
                        Alu.bypass,
                        replica_groups=[[0, 1], [2, 3], [4, 5], [6, 7]],
                        ins=[vin.opt()],
                        outs=[vout.opt()],
                    )
                    for t in range(KT):
                        vt = v_p.tile(
                            [P, 8 * (HD + 1)], bf16, tag="v", name=f"vx{g}_{t}"
                        )
                        nc.sync.dma_start(vt[:], vout[t * P:(t + 1) * P, :])
                        state["v_sb"][t] = vt
                steps.append(v_ag)
            else:
                for t in range(KT):
                    steps.append(vproj_step(t, v_p, state["v_sb"], t))
        return steps

    def _attention(g, pending):
        qt = state["qt"][g]
        kt = state["kt"][g]
        # snapshot: interleaved proj steps may swap in the NEXT superblock's
        # v tiles mid-loop; this group must keep reading its own
        v_sb = list(state["v_sb"])
        at = attn_p.tile([P, Q], bf16, tag="attn")
        for qr in range(2):
            q0 = qr * 512
            av = [
                ps_av.tile([P, 512], f32, tag="av", name=f"av{g}_{qr}_{i}")
                for i in range(2)
            ]
            for t in range(KT):
                if pending:
                    pending.popleft()()
                sc = ps_sc.tile([P, Q], f32, tag="sc")
                for h_loc in range(2):
                    r0 = h_loc * HD
                    nc.tensor.matmul(
                        sc[:, h_loc * 512:(h_loc + 1) * 512],
                        kt[r0:r0 + HD, t * P:(t + 1) * P],
                        qt[r0:r0 + HD, q0:q0 + 512],
                        start=True, stop=True,
                    )
                ex = exp_p.tile([P, Q], bf16, tag="ex")
                if t in dve_tset:
                    # Schraudolph: bf16 bits = round(sc*A_SCH + B_SCH)
                    nc.vector.tensor_scalar(
                        ex[:].bitcast(i16), sc[:], A_SCH, B_SCH,
                        Alu.mult, Alu.add,
                    )
                else:
                    nc.scalar.activation(ex[:], sc[:], Exp, scale=float(SCALE))
                for h_loc in range(2):
                    hs = ((2 * g + h_loc) % 8) * (HD + 1)
                    nc.tensor.matmul(
                        av[h_loc][0:HD + 1, :],
                        v_sb[t][:, hs:hs + HD + 1],
                        ex[:, h_loc * 512:(h_loc + 1) * 512],
                        start=(t == 0), stop=(t == KT - 1),
                    )
            # normalize by the denominator row (partition HD=64)
            for h_loc in range(2):
                rcp = rc_p.tile([P, 512], f32r, tag="rc")
                if CFG["nr_recip"]:
                    # y0n = bitcast(NOT(d_bits) + RC_C)  (~ -1/d)
                    # rcp = -(d*y0n + 2) * y0n           (= -1/d refined)
                    y0 = rc_p.tile([P, 512], f32, tag="y0")
                    nc.vector.tensor_scalar(
                        y0[HD:HD + 1, :].bitcast(i32),
                        av[h_loc][HD:HD + 1, :].bitcast(i32),
                        -1.0, float(RC_C - 1 - (1 << 32)),
                        Alu.mult, Alu.add,
                    )
                    t1 = rc_p.tile([P, 512], f32, tag="t1")
                    nc.vector.tensor_mul(
                        t1[HD:HD + 1, :], av[h_loc][HD:HD + 1, :], y0[HD:HD + 1, :]
                    )
                    nc.vector.scalar_tensor_tensor(
                        rcp[HD:HD + 1, :], t1[HD:HD + 1, :], 2.0,
                        y0[HD:HD + 1, :], Alu.add, Alu.mult,
                    )
                else:
                    with nc.allow_low_precision(reason="fp32r recip of softmax denom"):
                        nc.vector.reciprocal(
                            rcp[HD:HD + 1, :], av[h_loc][HD:HD + 1, :]
                        )
                bc = ps_pr.tile([P, 512], f32, tag="prj")
                nc.tensor.matmul(
                    bc[0:HD, :], ones_r[HD:HD + 1, 0:HD], rcp[HD:HD + 1, :],
                    start=True, stop=True,
                )
                bcs = bcb_p.tile([HD, 512], f32, tag="bcb")
                nc.vector.tensor_copy(bcs[:], bc[0:HD, :])
                if h_loc == 0:
                    nc.vector.tensor_mul(
                        at[0:HD, q0:q0 + 512], av[h_loc][0:HD, :], bcs[:]
                    )
                else:
                    odd_t = odd_p.tile([HD, 512], bf16, tag="odd")
                    nc.vector.tensor_mul(
                        odd_t[:], av[h_loc][0:HD, :], bcs[:]
                    )
                    nc.sync.dma_start(at[HD:P, q0:q0 + 512], odd_t[:])
        return at

    # ---- software-pipelined main loop: proj(g+1) interleaves attention(g) ----
    attn_sb = []
    pending = deque()
    for s in _proj_steps(0):
        s()
    for g in range(NG):
        if g + 1 < NG:
            pending.extend(_proj_steps(g + 1))
        if g == NG - 1:
            wo_pre = []

            def wo_prefetch():
                for dt in range(DT):
                    wr = wo_p.tile([P, 512], bf16, tag="wo", name=f"wopre{dt}")
                    nc.sync.dma_start(wr[:], woT_d[dt * P:(dt + 1) * P, 0:512])
                    wo_pre.append(wr)
            pending.append(wo_prefetch)
        attn_sb.append(_attention(g, pending))
        while pending:
            pending.popleft()()

    # ---- output projection: out[qr, dout] ----
    for nh in range(2):
        if nh == 0:
            wo_r = wo_pre
        else:
            wo_r = []
            for dt in range(DT):
                wr = wo_p.tile([P, 512], bf16, tag="wo")
                nc.sync.dma_start(
                    wr[:], woT_d[dt * P:(dt + 1) * P, nh * 512:(nh + 1) * 512]
                )
                wo_r.append(wr)
        for qrt in range(Q // P):
            ps = ps_pr.tile([P, 512], f32, tag="prj")
            for dt in range(DT):
                nc.tensor.matmul(
                    ps[:], attn_sb[dt][:, qrt * P:(qrt + 1) * P], wo_r[dt][:],
                    start=(dt == 0), stop=(dt == DT - 1),
                )
            ot = out_p.tile([P, 512], f32, tag="out")
            _evict("out", ot[:], ps[:])
            nc.sync.dma_start(
                out_d[qrt * P:(qrt + 1) * P, nh * 512:(nh + 1) * 512], ot[:]
            )
    return nc


def _get_exec(repeat=1):
    """Build the Bass module once and wrap it in a cached 8-core jitted callable."""
    key = ("exec", repeat)
    if key in _CACHE:
        return _CACHE[key]

    import jax
    import concourse.mybir as mybir
    from concourse import bass2jax
    from jax.experimental.shard_map import shard_map
    from jax.sharding import Mesh, PartitionSpec

    nc = _build_bass(repeat)
    bass2jax.install_neuronx_cc_hook()

    partition_name = nc.partition_id_tensor.name if nc.partition_id_tensor else None
    in_names, out_names, out_avals = [], [], []
    for alloc in nc.m.functions[0].allocations:
        if not isinstance(alloc, mybir.MemoryLocationSet):
            continue
        name = alloc.memorylocations[0].name
        if alloc.kind == "ExternalInput":
            if name != partition_name:
                in_names.append(name)
        elif alloc.kind == "ExternalOutput":
            out_names.append(name)
            out_avals.append(
                jax.core.ShapedArray(tuple(alloc.tensor_shape), mybir.dt.np(alloc.dtype))
            )
    n_params = len(in_names)
    all_names = in_names + out_names
    if partition_name is not None:
        all_names = all_names + [partition_name]

    def _body(*args):
        operands = list(args)
        if partition_name is not None:
            operands.append(bass2jax.partition_id_tensor())
        outs = bass2jax._bass_exec_p.bind(
            *operands,
            out_avals=tuple(out_avals),
            in_names=tuple(all_names),
            out_names=tuple(out_names),
            lowering_input_output_aliases=(),
            sim_require_finite=True,
            sim_require_nnan=True,
            nc=nc,
        )
        return tuple(outs)

    devices = jax.devices()[:8]
    mesh = Mesh(np.asarray(devices), ("core",))
    n_out = len(out_names)
    sharded = jax.jit(
        shard_map(
            _body,
            mesh=mesh,
            in_specs=(PartitionSpec("core"),) * (n_params + n_out),
            out_specs=(PartitionSpec("core"),) * n_out,
            check_rep=False,
        ),
        keep_unused=True,
    )
    _CACHE[("nc", repeat)] = nc
    _CACHE["nc"] = nc
    _CACHE[key] = (sharded, in_names, out_names, out_avals)
    return _CACHE[key]


def _to_fp32r(a):
    """Round fp32 to the fp32r grid: RNE at the low-12-mantissa-bit boundary
    (matches walrus fp32_to_fp32r: downconv to e8m11, stored <<12)."""
    u = np.ascontiguousarray(a, np.float32).view(np.uint32)
    low = u & np.uint32(0xFFF)
    base = u & ~np.uint32(0xFFF)
    round_up = (low > 0x800) | ((low == 0x800) & (((u >> 12) & 1) == 1))
    return (base + (round_up.astype(np.uint32) << 12)).view(np.float32)


def _prep_in_maps(query, WqT, WkT, WvT, WoT):
    import ml_dtypes

    WqTr, WkTr, WvTr = _to_fp32r(WqT), _to_fp32r(WkT), _to_fp32r(WvT)
    WoTb = np.ascontiguousarray(WoT).astype(ml_dtypes.bfloat16)
    in_maps = []
    for c in range(8):
        b, half = c // 2, c % 2
        xT = query[b].T
        if half == 1:
            xT = np.concatenate([xT[:, Q:], xT[:, :Q]], axis=1)
        in_maps.append({
            "xT": _to_fp32r(xT),
            "wqT": WqTr, "wkT": WkTr, "wvT": WvTr, "woT": WoTb,
        })
    return in_maps


def _run_device(in_maps):
    sharded, in_names, out_names, out_avals = _get_exec()
    concat_in = [
        np.concatenate([m[name] for m in in_maps], axis=0) for name in in_names
    ]
    zeros = [
        np.zeros((8 * a.shape[0], *a.shape[1:]), a.dtype) for a in out_avals
    ]
    out_arrs = sharded(*concat_in, *zeros)
    per_core = []
    for c in range(8):
        per_core.append({
            name: np.asarray(out_arrs[i]).reshape(8, *out_avals[i].shape)[c]
            for i, name in enumerate(out_names)
        })
    return per_core


def _numpy_fallback(query, Wq, bq, Wk, bk, Wv, bv, Wo, bo):
    q = query @ Wq.T + bq
    k = query @ Wk.T + bk
    v = query @ Wv.T + bv
    q = q.reshape(B, S, H, HD).transpose(0, 2, 1, 3)
    k = k.reshape(B, S, H, HD).transpose(0, 2, 1, 3)
    v = v.reshape(B, S, H, HD).transpose(0, 2, 1, 3)
    scores = np.einsum("bhqd,bhkd->bhqk", q, k) / np.sqrt(np.float32(HD))
    scores -= scores.max(axis=-1, keepdims=True)
    e = np.exp(scores)
    attn = e / e.sum(axis=-1, keepdims=True)
    out = np.einsum("bhqk,bhkd->bhqd", attn, v)
    out = out.transpose(0, 2, 1, 3).reshape(B, S, D)
    return (out @ Wo.T + bo).astype(np.float32)


def kernel(query, Wq, bq, Wk, bk, Wv, bv, Wo, bo):
    query = np.asarray(query, np.float32)
    Wq, Wk, Wv, Wo = (np.asarray(w, np.float32) for w in (Wq, Wk, Wv, Wo))
    bq, bk, bv, bo = (np.asarray(b_, np.float32) for b_ in (bq, bk, bv, bo))
    if any(np.any(b_) for b_ in (bq, bk, bv, bo)):
        return _numpy_fallback(query, Wq, bq, Wk, bk, Wv, bv, Wo, bo)

    WqT = np.ascontiguousarray(Wq.T)
    WkT = np.ascontiguousarray(Wk.T)
    WvT = np.ascontiguousarray(Wv.T)
    WoT = np.ascontiguousarray(Wo.T)
    in_maps = _prep_in_maps(query, WqT, WkT, WvT, WoT)
    per_core = _run_device(in_maps)
    out = np.empty((B, S, D), np.float32)
    for c in range(8):
        b, half = c // 2, c % 2
        out[b, half * Q:(half + 1) * Q] = per_core[c]["out"]
    return out


# revision 14
# speedup vs baseline: 1.9263x; 1.9263x over previous
"""Multi-head self-attention (B=4, S=2048, D=1024, H=16) on 8 trn2 NeuronCores.

Sharding: data-parallel over (batch, seq-half) -> 8 shards of 1024 query rows.
K/V halves are exchanged between the two cores of a batch via pairwise
AllGather collectives (each core projects K/V only for its own 1024 rows).

Per-core device kernel (Tile / fp32r matmuls, bf16 for AV + output proj):
  xT = query[b].T (host-transposed, rolled so this core's q rows are cols 0:1023)
  kT[g]  = (Wk x_own)^T : [128 dout, 1024 kr] own half, AllGather -> 2048 kr
  qT[g]  = (Wq x)^T   : [128 dout, 1024 qr]
  v      = x_own^T Wv^T : [kr, dout] + ones column per head (bf16), AllGather
  scores^T = kT^T qT  : [kr, qr] per (head, kr-tile)
  exp: ACT spline for most kr-tiles; Schraudolph bf16 bit-trick on DVE for
       CFG["dve_t"] of 16 tiles (offloads the ACT bottleneck)
  attn_outT/denom     = v_aug^T exp : [65, qr] PSUM (M=65: 64 hd + denom row)
  normalize: 3-op Newton reciprocal on DVE (int-seed + 1 NR), negated-ones
       outer-product broadcast on PE, DVE multiply
  out = attn_outT^T WoT -> [1024 qr, 1024 dout] fp32
"""
import sys

sys.path.insert(0, "/opt/trn_rl_repo")

import numpy as np

B, S, D, H = 4, 2048, 1024, 16
HD = D // H          # 64
P = 128
Q = S // 2           # q rows per core
NG = H // 2          # 8 head-pair groups
DT = D // P          # 8 din tiles
KT = S // P          # 16 kr tiles
SCALE = 1.0 / np.sqrt(np.float32(HD))  # 0.125

# Schraudolph exp -> bf16 bits: round(x*SCALE*128/ln2 + (127*128 - 7))
A_SCH = float(0.125 * 128.0 / np.log(2.0))
B_SCH = float(127.0 * 128.0 - 7.0)
# Newton reciprocal seed: bits = NOT(x_bits) + RC_C (== RC_C - 1 - x_bits)
RC_C = 0xFEF327EB  # (C1+1) for seed of -1/x

_CACHE = {}

# tunables
CFG = {
    "kt_bufs": 3, "qt_bufs": 3, "v_bufs": KT + 1, "exp_bufs": 4,
    "sc_bufs": 2, "av_bufs": 2, "pr_bufs": 2, "rc_bufs": 2,
    "v_inter": True,
    "kv_exchange": True,
    "dve_t": 4,           # how many of 16 kr-tiles use the DVE bit-trick exp
    "nr_recip": True,
    "act_evict": ("v", "out"),  # which psum evictions run on ACT
}
import os as _os, json as _json
if _os.environ.get("KCFG"):
    CFG.update(_json.loads(_os.environ["KCFG"]))


def _build_bass(repeat=1):
    import concourse.bass as bass
    import concourse.tile as tile
    from concourse import mybir
    from concourse.vector_clock import ScopedClock

    f32 = mybir.dt.float32
    f32r = mybir.dt.float32r
    bf16 = mybir.dt.bfloat16
    i32 = mybir.dt.int32
    i16 = mybir.dt.int16

    # This walrus build only accepts ONE sync-wait per CTRL instruction; the
    # stock Tile exit drain packs all outstanding sem waits onto a single
    # Drain. Spread them across sync-engine nops instead.
    def _drain_and_barrier(self, tick_clock, wait_clock):
        nc = self.nc
        drain_inst = nc.sync.drain()
        wait_clock.add_sem_waits(
            drain_inst.ins, ScopedClock({None: tick_clock.global_clock})
        )
        si = drain_inst.ins.sync_info
        waits = list(si.on_wait) if si is not None else []
        if len(waits) > 1:
            drain_inst.ins.sync_info = mybir.SyncInfo(
                on_wait=waits[:1], on_update=list(si.on_update)
            )
            for i in range(1, len(waits)):
                nop = nc.sync.nop(nofuse=True, hint="drain_wait_split")
                nop.ins.sync_info = mybir.SyncInfo(on_wait=[waits[i]], on_update=[])
        nc.all_engine_barrier()
        popped = nc._tile_sem_poison_stack.pop()
        assert popped is self._sem_poison
        nc.clear_and_free_semaphores(list(self.sems.allocated().values()))
        nc.all_engine_barrier()

    tile.TileContext._drain_and_barrier = _drain_and_barrier

    # Same walrus limitation, general case: any instruction may carry at most
    # one sync wait. Split extras onto same-engine nops placed just before.
    if not getattr(tile.TileContext, "_wait_split_patched", False):
        _orig_lower = tile.TileContext._lower_ordered_insts

        def _lower_with_wait_split(self, ordered):
            counter = 0
            for bb_name in list(ordered.keys()):
                new_insts = []
                for inst in ordered[bb_name]:
                    si = inst.sync_info
                    if si is not None and len(si.on_wait or []) > 1:
                        waits = list(si.on_wait)
                        for w in waits[:-1]:
                            counter += 1
                            nop = mybir.InstNoOp(
                                name=f"I-waitsplit-{bb_name}-{counter}", ins=[], outs=[]
                            )
                            nop.engine = inst.engine
                            nop.sync_info = mybir.SyncInfo(on_wait=[w], on_update=[])
                            new_insts.append(nop)
                        inst.sync_info = mybir.SyncInfo(
                            on_wait=[waits[-1]], on_update=list(si.on_update)
                        )
                    new_insts.append(inst)
                ordered[bb_name] = new_insts
            return _orig_lower(self, ordered)

        tile.TileContext._lower_ordered_insts = _lower_with_wait_split
        tile.TileContext._wait_split_patched = True

    Exp = mybir.ActivationFunctionType.Exp

    nc = bass.Bass()
    xT_d = nc.dram_tensor("xT", [D, S], f32r, kind="ExternalInput")
    wqT_d = nc.dram_tensor("wqT", [D, D], f32r, kind="ExternalInput")
    wkT_d = nc.dram_tensor("wkT", [D, D], f32r, kind="ExternalInput")
    wvT_d = nc.dram_tensor("wvT", [D, D], f32r, kind="ExternalInput")
    woT_d = nc.dram_tensor("woT", [D, D], bf16, kind="ExternalInput")
    out_d = nc.dram_tensor("out", [Q, D], f32, kind="ExternalOutput")

    from contextlib import ExitStack

    with tile.TileContext(nc) as tc:
        with ExitStack() as ctx:
            pool = lambda name, bufs, **kw: ctx.enter_context(
                tc.tile_pool(name=name, bufs=bufs, **kw)
            )
            xr_p = pool("xr", DT)            # xT fp32r resident
            wqk_p = pool("wqk", 10)          # per-group W tiles
            wv_p = pool("wv", 8)
            wo_p = pool("wo", 9)
            kt_p = pool("kt", CFG["kt_bufs"])
            kto_p = pool("kto", 2)           # own-half kT staging
            qt_p = pool("qt", CFG["qt_bufs"])
            v_p = pool("vv", CFG["v_bufs"])
            vo_p = pool("vo", 8)             # own-half v staging
            exp_p = pool("expp", CFG["exp_bufs"])
            attn_p = pool("attn", DT)
            odd_p = pool("odd", 2)
            rc_p = pool("rc", CFG["rc_bufs"])
            bcb_p = pool("bcb", 2)
            out_p = pool("outp", 2)
            ones_p = pool("ones", 1)
            dram_p = pool("dramb", 24, space="DRAM") if CFG["kv_exchange"] else None
            ps_sc = pool("ps_sc", CFG["sc_bufs"], space="PSUM")
            ps_av = pool("ps_av", CFG["av_bufs"], space="PSUM")
            ps_pr = pool("ps_pr", CFG["pr_bufs"], space="PSUM")
            # negated-ones row for the reciprocal-broadcast outer product
            # (NR recip produces -1/d; -1 * -1/d = +1/d)
            ones_f = ones_p.tile([P, HD], f32, tag="ones_f")
            nc.vector.memset(ones_f[:], -1.0 if CFG["nr_recip"] else 1.0)
            ones_r = ones_p.tile([P, HD], f32r, tag="ones_r")
            nc.vector.tensor_copy(ones_r[:], ones_f[:])
            seed_c = None  # reciprocal seed constants passed as immediates

            for _rep in range(repeat):
                _kernel_body(
                    nc, tc, mybir, f32, f32r, bf16, i32, i16, Exp,
                    xT_d, wqT_d, wkT_d, wvT_d, woT_d, out_d,
                    xr_p, wqk_p, wv_p, wo_p, kt_p, kto_p, qt_p, v_p, vo_p,
                    exp_p, attn_p, odd_p, rc_p, bcb_p, out_p, dram_p,
                    ones_r, seed_c, ps_sc, ps_av, ps_pr,
                )
    return nc


def _kernel_body(
    nc, tc, mybir, f32, f32r, bf16, i32, i16, Exp,
    xT_d, wqT_d, wkT_d, wvT_d, woT_d, out_d,
    xr_p, wqk_p, wv_p, wo_p, kt_p, kto_p, qt_p, v_p, vo_p,
    exp_p, attn_p, odd_p, rc_p, bcb_p, out_p, dram_p,
    ones_r, seed_c, ps_sc, ps_av, ps_pr,
):
    from collections import deque

    Alu = mybir.AluOpType
    exchange = CFG["kv_exchange"]
    dve_tset = set()
    if CFG["dve_t"] > 0:
        step = KT // CFG["dve_t"]
        dve_tset = {(i * step + step // 2) % KT for i in range(CFG["dve_t"])}

    def _evict(kind, dst, src):
        if kind in CFG["act_evict"]:
            nc.scalar.copy(dst, src)
        else:
            nc.vector.tensor_copy(dst, src)

    # load xT (pre-rounded fp32r bits) straight into resident tiles.
    # Own-half (cols 0:1024) first so own-half K/V projection starts early.
    xr = [xr_p.tile([P, S], f32r, tag="xr", name=f"xr{dt}") for dt in range(DT)]
    for ch in range(4):
        for dt in range(DT):
            nc.sync.dma_start(
                xr[dt][:, ch * 512:(ch + 1) * 512],
                xT_d[dt * P:(dt + 1) * P, ch * 512:(ch + 1) * 512],
            )

    state = {"v_sb": [None] * KT, "kt": {}, "qt": {}}

    def _proj_steps(g):
        """Emit-steps producing qt[g], kt[g] (+exchange) and v superblock."""
        steps = []
        wq_r, wk_r = [], []

        def load_w():
            for dt in range(DT):
                for w_d, lst, tg in ((wqT_d, wq_r, "wq"), (wkT_d, wk_r, "wk")):
                    wr = wqk_p.tile([P, P], f32r, tag=tg)
                    nc.sync.dma_start(
                        wr[:], w_d[dt * P:(dt + 1) * P, g * P:(g + 1) * P]
                    )
                    lst.append(wr)
        steps.append(load_w)

        qt = qt_p.tile([P, Q], bf16, tag="qt", name=f"qt{g}")
        state["qt"][g] = qt

        def qt_step(n):
            def _s():
                ps = ps_pr.tile([P, 512], f32, tag="prj")
                for dt in range(DT):
                    nc.tensor.matmul(
                        ps[:], wq_r[dt][:], xr[dt][:, n * 512:(n + 1) * 512],
                        start=(dt == 0), stop=(dt == DT - 1),
                    )
                _evict("qt", qt[:, n * 512:(n + 1) * 512], ps[:])
            return _s
        steps += [qt_step(0), qt_step(1)]

        kt = kt_p.tile([P, S], bf16, tag="kt", name=f"kt{g}")
        state["kt"][g] = kt
        if exchange:
            kto = kto_p.tile([P, Q], bf16, tag="kto", name=f"kto{g}")

            def kt_step(n):
                def _s():
                    ps = ps_pr.tile([P, 512], f32, tag="prj")
                    for dt in range(DT):
                        nc.tensor.matmul(
                            ps[:], wk_r[dt][:], xr[dt][:, n * 512:(n + 1) * 512],
                            start=(dt == 0), stop=(dt == DT - 1),
                        )
                    _evict("kt", kto[:, n * 512:(n + 1) * 512], ps[:])
                return _s

            def kt_ag():
                kin = dram_p.tile([P, Q], bf16, tag="kin", name=f"kin{g}")
                kout = dram_p.tile([2 * P, Q], bf16, tag="kout", name=f"kout{g}")
                nc.gpsimd.dma_start(kin[:], kto[:])
                nc.gpsimd.collective_compute(
                    "AllGather",
                    Alu.bypass,
                    replica_groups=[[0, 1], [2, 3], [4, 5], [6, 7]],
                    ins=[kin.opt()],
                    outs=[kout.opt()],
                )
                nc.sync.dma_start(kt[:, 0:Q], kout[0:P, :])
                nc.sync.dma_start(kt[:, Q:S], kout[P:2 * P, :])
            steps += [kt_step(0), kt_step(1), kt_ag]
        else:
            def kt_step(n):
                def _s():
                    ps = ps_pr.tile([P, 512], f32, tag="prj")
                    for dt in range(DT):
                        nc.tensor.matmul(
                            ps[:], wk_r[dt][:], xr[dt][:, n * 512:(n + 1) * 512],
                            start=(dt == 0), stop=(dt == DT - 1),
                        )
                    _evict("kt", kt[:, n * 512:(n + 1) * 512], ps[:])
                return _s
            steps += [kt_step(n) for n in range(4)]

        # V projection for superblock of 8 heads (every 4 groups)
        if g % 4 == 0:
            sbi = g // 4
            wv_r = []

            def load_wv():
                for dt in range(DT):
                    wr = wv_p.tile([P, 512], f32r, tag="wv")
                    nc.sync.dma_start(
                        wr[:],
                        wvT_d[dt * P:(dt + 1) * P, sbi * 512:(sbi + 1) * 512],
                    )
                    wv_r.append(wr)
            steps.append(load_wv)

            def vproj_step(t, dst_pool, dst_list, idx):
                def _s():
                    ps = ps_pr.tile([P, 512], f32, tag="prj", name=f"vps{g}_{t}")
                    for dt in range(DT):
                        nc.tensor.matmul(
                            ps[:], xr[dt][:, t * P:(t + 1) * P], wv_r[dt][:],
                            start=(dt == 0), stop=(dt == DT - 1),
                        )
                    vt = dst_pool.tile(
                        [P, 8 * (HD + 1)], bf16, tag="v", name=f"v{g}_{t}"
                    )
                    dst = vt[:].rearrange("p (h c) -> p h c", c=HD + 1)
                    src = ps[:].rearrange("p (h c) -> p h c", c=HD)
                    _evict("v", dst[:, :, 0:HD], src[:])
                    nc.vector.memset(dst[:, :, HD:HD + 1], 1.0)
                    dst_list[idx] = vt
                return _s

            if exchange:
                vown = [None] * (KT // 2)
                for tl in range(KT // 2):
                    steps.append(vproj_step(tl, vo_p, vown, tl))

                def v_ag():
                    vin = dram_p.tile(
                        [Q, 8 * (HD + 1)], bf16, tag="vin", name=f"vin{sbi}"
                    )
                    vout = dram_p.tile(
                        [S, 8 * (HD + 1)], bf16, tag="vout", name=f"vout{sbi}"
                    )
                    for tl in range(KT // 2):
                        nc.gpsimd.dma_start(
                            vin[tl * P:(tl + 1) * P, :], vown[tl][:]
                        )
                    nc.gpsimd.collective_compute(
                        "AllGather",
                        Alu.bypass,
                        replica_groups=[[0, 1], [2, 3], [4, 5], [6, 7]],
                        ins=[vin.opt()],
                        outs=[vout.opt()],
                    )
                    for t in range(KT):
                        vt = v_p.tile(
                            [P, 8 * (HD + 1)], bf16, tag="v", name=f"vx{g}_{t}"
                        )
                        nc.sync.dma_start(vt[:], vout[t * P:(t + 1) * P, :])
                        state["v_sb"][t] = vt
                steps.append(v_ag)
            else:
                # no-exchange: V is projected inline during attention (the
                # queued form deadlocks the evict engine FIFO against the
                # previous superblock's readers)
                state["wv_r"] = wv_r
                state["vproj_fn"] = vproj_step
        return steps

    def _attention(g, pending):
        qt = state["qt"][g]
        kt = state["kt"][g]
        # snapshot: interleaved proj steps may swap in the NEXT superblock's
        # v tiles mid-loop; this group must keep reading its own
        v_sb = list(state["v_sb"])
        at = attn_p.tile([P, Q], bf16, tag="attn")
        for qr in range(2):
            q0 = qr * 512
            av = [
                ps_av.tile([P, 512], f32, tag="av", name=f"av{g}_{qr}_{i}")
                for i in range(2)
            ]
            for t in range(KT):
                if (not exchange) and g % 4 == 0 and qr == 0:
                    state["vproj_fn"](t, v_p, state["v_sb"], t)()
                    v_sb[t] = state["v_sb"][t]
                if pending:
                    pending.popleft()()
                sc = ps_sc.tile([P, Q], f32, tag="sc")
                for h_loc in range(2):
                    r0 = h_loc * HD
                    nc.tensor.matmul(
                        sc[:, h_loc * 512:(h_loc + 1) * 512],
                        kt[r0:r0 + HD, t * P:(t + 1) * P],
                        qt[r0:r0 + HD, q0:q0 + 512],
                        start=True, stop=True,
                    )
                ex = exp_p.tile([P, Q], bf16, tag="ex")
                if t in dve_tset:
                    # Schraudolph: bf16 bits = round(sc*A_SCH + B_SCH)
                    nc.vector.tensor_scalar(
                        ex[:].bitcast(i16), sc[:], A_SCH, B_SCH,
                        Alu.mult, Alu.add,
                    )
                else:
                    nc.scalar.activation(ex[:], sc[:], Exp, scale=float(SCALE))
                for h_loc in range(2):
                    hs = ((2 * g + h_loc) % 8) * (HD + 1)
                    nc.tensor.matmul(
                        av[h_loc][0:HD + 1, :],
                        v_sb[t][:, hs:hs + HD + 1],
                        ex[:, h_loc * 512:(h_loc + 1) * 512],
                        start=(t == 0), stop=(t == KT - 1),
                    )
            # normalize by the denominator row (partition HD=64)
            for h_loc in range(2):
                rcp = rc_p.tile([P, 512], f32r, tag="rc")
                if CFG["nr_recip"]:
                    # y0n = bitcast(NOT(d_bits) + RC_C)  (~ -1/d)
                    # rcp = -(d*y0n + 2) * y0n           (= -1/d refined)
                    y0 = rc_p.tile([P, 512], f32, tag="y0")
                    nc.vector.tensor_scalar(
                        y0[HD:HD + 1, :].bitcast(i32),
                        av[h_loc][HD:HD + 1, :].bitcast(i32),
                        -1.0, float(RC_C - 1 - (1 << 32)),
                        Alu.mult, Alu.add,
                    )
                    t1 = rc_p.tile([P, 512], f32, tag="t1")
                    nc.vector.tensor_mul(
                        t1[HD:HD + 1, :], av[h_loc][HD:HD + 1, :], y0[HD:HD + 1, :]
                    )
                    nc.vector.scalar_tensor_tensor(
                        rcp[HD:HD + 1, :], t1[HD:HD + 1, :], 2.0,
                        y0[HD:HD + 1, :], Alu.add, Alu.mult,
                    )
                else:
                    with nc.allow_low_precision(reason="fp32r recip of softmax denom"):
                        nc.vector.reciprocal(
                            rcp[HD:HD + 1, :], av[h_loc][HD:HD + 1, :]
                        )
                bc = ps_pr.tile([P, 512], f32, tag="prj")
                nc.tensor.matmul(
                    bc[0:HD, :], ones_r[HD:HD + 1, 0:HD], rcp[HD:HD + 1, :],
                    start=True, stop=True,
                )
                bcs = bcb_p.tile([HD, 512], f32, tag="bcb")
                nc.vector.tensor_copy(bcs[:], bc[0:HD, :])
                if h_loc == 0:
                    nc.vector.tensor_mul(
                        at[0:HD, q0:q0 + 512], av[h_loc][0:HD, :], bcs[:]
                    )
                else:
                    odd_t = odd_p.tile([HD, 512], bf16, tag="odd")
                    nc.vector.tensor_mul(
                        odd_t[:], av[h_loc][0:HD, :], bcs[:]
                    )
                    nc.sync.dma_start(at[HD:P, q0:q0 + 512], odd_t[:])
        return at

    # ---- software-pipelined main loop: proj(g+1) interleaves attention(g) ----
    attn_sb = []
    pending = deque()
    for s in _proj_steps(0):
        s()
    if exchange:
        # depth-2 prefetch: proj(1) runs eagerly (fills the first AllGather's
        # latency with PE work); proj(g+2) interleaves attention(g)
        for s in _proj_steps(1):
            s()
    for g in range(NG):
        g_next = g + 2 if exchange else g + 1
        if g_next < NG and (exchange or True) and (g_next > 1 or not exchange):
            pending.extend(_proj_steps(g_next))
        if g == NG - 1:
            wo_pre = []

            def wo_prefetch():
                for dt in range(DT):
                    wr = wo_p.tile([P, 512], bf16, tag="wo", name=f"wopre{dt}")
                    nc.sync.dma_start(wr[:], woT_d[dt * P:(dt + 1) * P, 0:512])
                    wo_pre.append(wr)
            pending.append(wo_prefetch)
        attn_sb.append(_attention(g, pending))
        while pending:
            pending.popleft()()

    # ---- output projection: out[qr, dout] ----
    for nh in range(2):
        if nh == 0:
            wo_r = wo_pre
        else:
            wo_r = []
            for dt in range(DT):
                wr = wo_p.tile([P, 512], bf16, tag="wo")
                nc.sync.dma_start(
                    wr[:], woT_d[dt * P:(dt + 1) * P, nh * 512:(nh + 1) * 512]
                )
                wo_r.append(wr)
        for qrt in range(Q // P):
            ps = ps_pr.tile([P, 512], f32, tag="prj")
            for dt in range(DT):
                nc.tensor.matmul(
                    ps[:], attn_sb[dt][:, qrt * P:(qrt + 1) * P], wo_r[dt][:],
                    start=(dt == 0), stop=(dt == DT - 1),
                )
            ot = out_p.tile([P, 512], f32, tag="out")
            _evict("out", ot[:], ps[:])
            nc.sync.dma_start(
                out_d[qrt * P:(qrt + 1) * P, nh * 512:(nh + 1) * 512], ot[:]
            )
    return nc


def _get_exec(repeat=1):
    """Build the Bass module once and wrap it in a cached 8-core jitted callable."""
    key = ("exec", repeat)
    if key in _CACHE:
        return _CACHE[key]

    import jax
    import concourse.mybir as mybir
    from concourse import bass2jax
    from jax.experimental.shard_map import shard_map
    from jax.sharding import Mesh, PartitionSpec

    nc = _build_bass(repeat)
    bass2jax.install_neuronx_cc_hook()

    partition_name = nc.partition_id_tensor.name if nc.partition_id_tensor else None
    in_names, out_names, out_avals = [], [], []
    for alloc in nc.m.functions[0].allocations:
        if not isinstance(alloc, mybir.MemoryLocationSet):
            continue
        name = alloc.memorylocations[0].name
        if alloc.kind == "ExternalInput":
            if name != partition_name:
                in_names.append(name)
        elif alloc.kind == "ExternalOutput":
            out_names.append(name)
            out_avals.append(
                jax.core.ShapedArray(tuple(alloc.tensor_shape), mybir.dt.np(alloc.dtype))
            )
    n_params = len(in_names)
    all_names = in_names + out_names
    if partition_name is not None:
        all_names = all_names + [partition_name]

    def _body(*args):
        operands = list(args)
        if partition_name is not None:
            operands.append(bass2jax.partition_id_tensor())
        outs = bass2jax._bass_exec_p.bind(
            *operands,
            out_avals=tuple(out_avals),
            in_names=tuple(all_names),
            out_names=tuple(out_names),
            lowering_input_output_aliases=(),
            sim_require_finite=True,
            sim_require_nnan=True,
            nc=nc,
        )
        return tuple(outs)

    devices = jax.devices()[:8]
    mesh = Mesh(np.asarray(devices), ("core",))
    n_out = len(out_names)
    sharded = jax.jit(
        shard_map(
            _body,
            mesh=mesh,
            in_specs=(PartitionSpec("core"),) * (n_params + n_out),
            out_specs=(PartitionSpec("core"),) * n_out,
            check_rep=False,
        ),
        keep_unused=True,
    )
    _CACHE[("nc", repeat)] = nc
    _CACHE["nc"] = nc
    _CACHE[key] = (sharded, in_names, out_names, out_avals)
    return _CACHE[key]


def _to_fp32r(a):
    """Round fp32 to the fp32r grid: RNE at the low-12-mantissa-bit boundary
    (matches walrus fp32_to_fp32r: downconv to e8m11, stored <<12)."""
    u = np.ascontiguousarray(a, np.float32).view(np.uint32)
    low = u & np.uint32(0xFFF)
    base = u & ~np.uint32(0xFFF)
    round_up = (low > 0x800) | ((low == 0x800) & (((u >> 12) & 1) == 1))
    return (base + (round_up.astype(np.uint32) << 12)).view(np.float32)


def _prep_in_maps(query, WqT, WkT, WvT, WoT):
    import ml_dtypes

    WqTr, WkTr, WvTr = _to_fp32r(WqT), _to_fp32r(WkT), _to_fp32r(WvT)
    WoTb = np.ascontiguousarray(WoT).astype(ml_dtypes.bfloat16)
    in_maps = []
    for c in range(8):
        b, half = c // 2, c % 2
        xT = query[b].T
        if half == 1:
            xT = np.concatenate([xT[:, Q:], xT[:, :Q]], axis=1)
        in_maps.append({
            "xT": _to_fp32r(xT),
            "wqT": WqTr, "wkT": WkTr, "wvT": WvTr, "woT": WoTb,
        })
    return in_maps


def _run_device(in_maps):
    sharded, in_names, out_names, out_avals = _get_exec()
    concat_in = [
        np.concatenate([m[name] for m in in_maps], axis=0) for name in in_names
    ]
    zeros = [
        np.zeros((8 * a.shape[0], *a.shape[1:]), a.dtype) for a in out_avals
    ]
    out_arrs = sharded(*concat_in, *zeros)
    per_core = []
    for c in range(8):
        per_core.append({
            name: np.asarray(out_arrs[i]).reshape(8, *out_avals[i].shape)[c]
            for i, name in enumerate(out_names)
        })
    return per_core


def _numpy_fallback(query, Wq, bq, Wk, bk, Wv, bv, Wo, bo):
    q = query @ Wq.T + bq
    k = query @ Wk.T + bk
    v = query @ Wv.T + bv
    q = q.reshape(B, S, H, HD).transpose(0, 2, 1, 3)
    k = k.reshape(B, S, H, HD).transpose(0, 2, 1, 3)
    v = v.reshape(B, S, H, HD).transpose(0, 2, 1, 3)
    scores = np.einsum("bhqd,bhkd->bhqk", q, k) / np.sqrt(np.float32(HD))
    scores -= scores.max(axis=-1, keepdims=True)
    e = np.exp(scores)
    attn = e / e.sum(axis=-1, keepdims=True)
    out = np.einsum("bhqk,bhkd->bhqd", attn, v)
    out = out.transpose(0, 2, 1, 3).reshape(B, S, D)
    return (out @ Wo.T + bo).astype(np.float32)


def kernel(query, Wq, bq, Wk, bk, Wv, bv, Wo, bo):
    query = np.asarray(query, np.float32)
    Wq, Wk, Wv, Wo = (np.asarray(w, np.float32) for w in (Wq, Wk, Wv, Wo))
    bq, bk, bv, bo = (np.asarray(b_, np.float32) for b_ in (bq, bk, bv, bo))
    if any(np.any(b_) for b_ in (bq, bk, bv, bo)):
        return _numpy_fallback(query, Wq, bq, Wk, bk, Wv, bv, Wo, bo)

    WqT = np.ascontiguousarray(Wq.T)
    WkT = np.ascontiguousarray(Wk.T)
    WvT = np.ascontiguousarray(Wv.T)
    WoT = np.ascontiguousarray(Wo.T)
    in_maps = _prep_in_maps(query, WqT, WkT, WvT, WoT)
    per_core = _run_device(in_maps)
    out = np.empty((B, S, D), np.float32)
    for c in range(8):
        b, half = c // 2, c % 2
        out[b, half * Q:(half + 1) * Q] = per_core[c]["out"]
    return out
